# revision 33
# baseline (speedup 1.0000x reference)
"""Multi-head causal attention (B=4, S=2048, D=1024, H=16) on 8 trn2 cores.

Sharding: (batch x head-group) grid -> core c handles batch c//2, heads
[8*(c%2), 8*(c%2)+8).  Each core computes q/k/v projections for its 512
qkv dims, attention for its 8 heads, and a partial output projection.
Host sums the two partial outputs per batch and adds bo.

v2 design (vs the f32r v1 baseline):
  - All matmul operands are bf16.  f32r matmuls must self-load their
    stationary operand on every MATMUL (standalone LDWEIGHTS is broken
    for f32/f32r in walrus), and fp32 weight loads don't get Fast
    Weight Load; the v1 trace showed 262us of LDWEIGHTS and a PE that
    HAM-throttled to 1.2GHz for ~250us of the 585us span.  bf16 gets
    FWL (4x faster loads, overlapped via the PE reorder window), halves
    DMA, and keeps the MM stream dense so HAM stays at 2.4GHz.
  - x is loaded once (bf16, 4MB resident) instead of twice.
  - Score matmuls for the two heads of a head-pair are emitted
    back-to-back as row-tiled (64x128) pairs at tile positions (0,0)
    and (64,0), writing the two banks of one [128,1024] PSUM tile; the
    PE runs them concurrently (contraction is only DH=64).
  - One exp op covers both heads ([128, 2, 512-c0] strided AP over the
    2-bank score tile) -> half the ACT per-op overhead.  v1 spent 220us
    on 320 exp ops (352-cycle fixed cost each).
  - Attention is emitted per q-block with scores/exp batched ahead of
    the attended MMs (sub-batches of 8 mk) to limit (64x128)<->(128x128)
    tile-mode switches.
  - att PSUM banks are freed immediately by copying [65,512] to SBUF;
    the softmax normalization (reciprocal-of-denominator broadcast)
    happens from SBUF afterwards, so "att" needs only 2 banks and the
    whole kernel fits in one flat 8-bank PSUM plan:
        pqkv (2) + scores (2x2) + att (2)
    which lets qkv / attention / outproj matmuls interleave freely.
  - Emission order interleaves qkv m-tiles with attention head-pairs
    (qk m=0, v, attn hp=0, qk m=1, attn hp=1, ...) so the scheduler can
    fill PE time during ACT-bound attention stretches with projection
    matmuls.
  - k-bias is dropped (cancels in softmax); q-bias applied via DVE
    tensor_scalar add during PSUM evacuation; v-bias via a replicated
    tile; o-bias added on host.  Softmax denominator comes from a
    ones-column appended to each head's V tile (M=65 stationary).
  - causal masking: (128k x 512q) blocks above the diagonal are
    skipped; diagonal tiles get an additive 0/-1e30 triangle (both
    heads in one DVE op via a [128,256] doubled-triangle tile).
"""

import os
import sys

import numpy as np

sys.path.insert(0, "/opt/trn_rl_repo")

from contextlib import ExitStack

import bass_rust

import concourse.bass as bass
import concourse.mybir as mybir
import concourse.tile as tile

# ---------------------------------------------------------------------------
# Compat shims for bass_rust (new) vs neuronxcc walrus (2026-05-04) skew:
#  1. Tile's epilogue emits EVENT_SEMAPHORE_RANGE_CLEAR (InstISA 176) which
#     this walrus rejects ("ISA wrong length") -> skip it.
#  2. This walrus supports only ONE sync-wait command per instruction; Tile
#     attaches several -> post-pass hoists extra waits onto NOPs inserted
#     just before, on the same engine.
# ---------------------------------------------------------------------------
_MAXW = 1


def _sem_ranges(nums):
    nums = sorted(nums)
    out = []
    start = prev = nums[0]
    for n in nums[1:]:
        if n == prev + 1:
            prev = n
            continue
        out.append(range(start, prev + 1))
        start = prev = n
    out.append(range(start, prev + 1))
    return out


def _install_compat():
    if getattr(bass, "_mha_compat_installed", False):
        return
    bass._mha_compat_installed = True
    from concourse.bass import SemaphoreHandle

    def clear_and_free_semaphores(self, sems):
        if not sems:
            return
        sem_nums = [s.num if isinstance(s, SemaphoreHandle) else s for s in sems]
        for r in _sem_ranges(sem_nums):
            assert self._state.free_isdisjoint(r)
            self.gpsimd.dma_reset(r)
            # skip sem_clear (ISA 176): unsupported by this walrus
        self._state.prepend_free_semaphores(sem_nums)
        for poison_set in self._tile_sem_poison_stack:
            poison_set.update(sem_nums)

    bass.Bass.clear_and_free_semaphores = clear_and_free_semaphores


def _split_sync_waits(nc):
    """Hoist extra sync waits (>_MAXW per instruction) onto NOP carriers."""

    def new_nop(engine):
        binst = nc.engines[engine].isa(
            nc.isa.Opcode.NEURON_ISA_TPB_OPCODE_NOP, {}
        )
        inst = binst.ins
        bb = nc.cur_bb.bb
        assert bb.instructions and bb.instructions[-1] is inst
        bb.instructions.pop()
        return inst

    for func in nc.m.functions:
        for blk in func.blocks:
            snapshot = list(blk.instructions)
            if not any(
                i.sync_info and i.sync_info.on_wait and len(i.sync_info.on_wait) > _MAXW
                for i in snapshot
            ):
                continue
            new = []
            for inst in snapshot:
                si = inst.sync_info
                waits = list(si.on_wait) if si and si.on_wait else []
                if len(waits) > _MAXW:
                    for w in waits[:-_MAXW]:
                        nop = new_nop(inst.engine)
                        nop.sync_info = bass_rust.SyncInfo(on_wait=[w], on_update=[])
                        new.append(nop)
                    upd = list(si.on_update) if si and si.on_update else []
                    inst.sync_info = bass_rust.SyncInfo(
                        on_wait=waits[-_MAXW:], on_update=upd
                    )
                new.append(inst)
            blk.instructions[:] = new

P = 128
S = 2048
D = 1024          # model dim (contraction for qkv / full e for out)
EL = 512          # per-core qkv width (8 heads * 64)
NH = 8            # local heads
DH = 64
NCORES = 8
SCALE = 1.0 / 8.0  # 1/sqrt(DH)
NEG = -1.0e30

ST = S // P       # 16 s-tiles
DT = D // P       # 8 d-tiles
ET = EL // P      # 4 local e-tiles (head pairs)
QB = 4            # q-blocks of 512
EXCHUNK = 8       # mk sub-batch between tile-mode switches
EXBUFS = 18

F32 = mybir.dt.float32
BF16 = mybir.dt.bfloat16

_PROGRAM_CACHE = {}


def build_program(mode, split_waits=True):
    """mode: 'causal' (tril mask) or 'full' (no masking).

    split_waits: apply the walrus one-wait-per-instruction post-pass (the
    NOP carriers it inserts are not understood by CoreSim, so simulation
    runs build with split_waits=False)."""
    assert mode in ("causal", "full")
    _install_compat()
    nc = bass.Bass("TRN2", target_bir_lowering=False, debug=False)

    xt_d = nc.dram_tensor("xt", [D, S], BF16, kind="ExternalInput").ap()
    wqt_d = nc.dram_tensor("wqt", [D, EL], BF16, kind="ExternalInput").ap()
    wkt_d = nc.dram_tensor("wkt", [D, EL], BF16, kind="ExternalInput").ap()
    wvt_d = nc.dram_tensor("wvt", [D, EL], BF16, kind="ExternalInput").ap()
    wot_d = nc.dram_tensor("wot", [EL, D], BF16, kind="ExternalInput").ap()
    bq_d = nc.dram_tensor("bq", [EL, 1], F32, kind="ExternalInput").ap()
    bvrep_d = nc.dram_tensor("bvrep", [P, EL], BF16, kind="ExternalInput").ap()
    tri2_d = nc.dram_tensor("tri2", [P, 2 * P], F32, kind="ExternalInput").ap()
    out_d = nc.dram_tensor("out", [S, D], F32, kind="ExternalOutput").ap()

    causal = mode == "causal"
    Exp = mybir.ActivationFunctionType.Exp

    with ExitStack() as ctx:
        tc = ctx.enter_context(tile.TileContext(nc))
        consts = ctx.enter_context(tc.tile_pool(name="consts", bufs=1))
        wpool = ctx.enter_context(tc.tile_pool(name="w", bufs=1))
        xpool = ctx.enter_context(tc.tile_pool(name="x", bufs=1))
        qkvp = ctx.enter_context(tc.tile_pool(name="qkv", bufs=1))
        attp = ctx.enter_context(tc.tile_pool(name="attsb", bufs=1))
        expp = ctx.enter_context(tc.tile_pool(name="exp", bufs=EXBUFS))
        attup = ctx.enter_context(tc.tile_pool(name="attu", bufs=4))
        smallp = ctx.enter_context(tc.tile_pool(name="small", bufs=4))
        dramp = ctx.enter_context(tc.tile_pool(name="dram", bufs=4, space="DRAM"))
        outp = ctx.enter_context(tc.tile_pool(name="outsb", bufs=3))
        psum = ctx.enter_context(tc.tile_pool(name="ps", bufs=1, space="PSUM"))

        # ---- input DMAs, ordered to minimize time-to-first-matmul:
        # qk(m=0) needs bq + wq + x chunk 0; v needs wv + bvrep; the rest
        # (wk is consumed alongside wq, remaining x chunks, tri2, wot)
        # streams behind the first compute.
        wv_sb = [wpool.tile([P, EL], BF16, tag=f"wv{k}", name=f"wv{k}") for k in range(DT)]
        wq_sb = [wpool.tile([P, EL], BF16, tag=f"wq{k}", name=f"wq{k}") for k in range(DT)]
        wk_sb = [wpool.tile([P, EL], BF16, tag=f"wk{k}", name=f"wk{k}") for k in range(DT)]
        xt_sb = [xpool.tile([P, S], BF16, tag=f"xt{k}", name=f"xt{k}") for k in range(DT)]
        # DMA order follows the critical path to the first exp: the whole
        # attention cascade (qk m=0 -> evac -> scores -> tri -> exp) is gated
        # by x/wk/tri arrival, so those stream first; wv/bvrep (v), wq m=1..3
        # and wot are consumed much later.
        bq_sb = consts.tile([P, ET], F32)
        for m in range(ET):
            nc.sync.dma_start(bq_sb[:, m : m + 1], bq_d[m * P : (m + 1) * P, :])
        for k in range(DT):
            nc.sync.dma_start(wq_sb[k][:, 0:P], wqt_d[k * P : (k + 1) * P, 0:P])
            nc.sync.dma_start(xt_sb[k][:, 0:512], xt_d[k * P : (k + 1) * P, 0:512])
        for k in range(DT):
            nc.sync.dma_start(wk_sb[k][:], wkt_d[k * P : (k + 1) * P, :])
        if causal:
            tri2_sb = consts.tile([P, 2 * P], F32)
            nc.sync.dma_start(tri2_sb[:], tri2_d)
        for k in range(DT):
            nc.sync.dma_start(xt_sb[k][:, 512:1024], xt_d[k * P : (k + 1) * P, 512:1024])
        for k in range(DT):
            nc.sync.dma_start(xt_sb[k][:, 1024:1536], xt_d[k * P : (k + 1) * P, 1024:1536])
        for k in range(DT):
            nc.sync.dma_start(xt_sb[k][:, 1536:2048], xt_d[k * P : (k + 1) * P, 1536:2048])
        for k in range(DT):
            nc.sync.dma_start(wv_sb[k][:], wvt_d[k * P : (k + 1) * P, :])
        bvrep_sb = consts.tile([P, EL], BF16)
        nc.sync.dma_start(bvrep_sb[:], bvrep_d)
        for k in range(DT):
            nc.sync.dma_start(wq_sb[k][:, P:EL], wqt_d[k * P : (k + 1) * P, P:EL])
        wot_sb = [wpool.tile([P, D], BF16, tag=f"wo{kt}", name=f"wo{kt}") for kt in range(ET)]
        for kt in range(ET):
            nc.sync.dma_start(wot_sb[kt][:], wot_d[kt * P : (kt + 1) * P, :])

        # ---- qkv outputs + attention result ----
        qt_sb = [qkvp.tile([P, S], BF16, tag=f"qt{m}", name=f"qt{m}") for m in range(ET)]
        kt_sb = [qkvp.tile([P, S], BF16, tag=f"kt{m}", name=f"kt{m}") for m in range(ET)]
        v_sb = [qkvp.tile([P, NH * (DH + 1)], BF16, tag=f"v{st}", name=f"v{st}") for st in range(ST)]
        att_sb = [attp.tile([P, S], BF16, tag=f"att{kt}", name=f"attsb{kt}") for kt in range(ET)]

        def emit_qk(m):
            for sc in range(4):
                s0 = sc * 512
                pq = psum.tile([P, 512], F32, tag="pqkv", bufs=2)
                for k in range(DT):
                    nc.tensor.matmul(
                        pq[:],
                        wq_sb[k][:, m * P : (m + 1) * P],
                        xt_sb[k][:, s0 : s0 + 512],
                        start=(k == 0),
                        stop=(k == DT - 1),
                    )
                nc.vector.tensor_scalar_add(
                    qt_sb[m][:, s0 : s0 + 512], pq[:], bq_sb[:, m : m + 1]
                )
                pk = psum.tile([P, 512], F32, tag="pqkv", bufs=2)
                for k in range(DT):
                    nc.tensor.matmul(
                        pk[:],
                        wk_sb[k][:, m * P : (m + 1) * P],
                        xt_sb[k][:, s0 : s0 + 512],
                        start=(k == 0),
                        stop=(k == DT - 1),
                    )
                nc.vector.tensor_copy(kt_sb[m][:, s0 : s0 + 512], pk[:])

        def emit_v():
            # pv uses the "att" psum slots (idle until the first attended
            # accumulation, which needs v anyway) so v never contends with
            # qk m=0's pqkv slots -- that contention delayed qt/kt
            # evacuation and with it the whole first-exp cascade.
            for st in range(ST):
                pv = psum.tile([P, EL], F32, tag="att", bufs=2)
                for k in range(DT):
                    nc.tensor.matmul(
                        pv[:],
                        xt_sb[k][:, st * P : (st + 1) * P],
                        wv_sb[k][:],
                        start=(k == 0),
                        stop=(k == DT - 1),
                    )
                vdst = v_sb[st][:].rearrange("p (h c) -> p h c", c=DH + 1)
                nc.vector.tensor_add(
                    vdst[:, :, 0:DH],
                    pv[:].rearrange("p (h c) -> p h c", c=DH),
                    bvrep_sb[:].rearrange("p (h c) -> p h c", c=DH),
                )
                nc.vector.memset(vdst[:, :, DH : DH + 1], 1.0)

        def hi_of(qbl):
            return 4 * qbl + 4 if causal else ST

        def alloc_att_ps(hp, qbl):
            return {
                hl: psum.tile([P, 512], F32, tag="att", bufs=2, name=f"attps{hp}{qbl}{hl}")
                for hl in (0, 1)
            }

        def emit_scores_exp(hp, qbl, mks):
            # scores (row-tiled head pairs) + mask + exp
            qb0 = qbl * 512
            exs = []
            for mk in mks:
                k0 = mk * P
                c0 = max(0, k0 - qb0) if causal else 0
                sp = psum.tile([P, 1024], F32, tag="sc", bufs=2)
                for hl in (0, 1):
                    nc.tensor.matmul(
                        sp[:, hl * 512 + c0 : hl * 512 + 512],
                        kt_sb[hp][hl * DH : (hl + 1) * DH, k0 : k0 + P],
                        qt_sb[hp][hl * DH : (hl + 1) * DH, qb0 + c0 : qb0 + 512],
                        start=True,
                        stop=True,
                    )
                spv = sp[:].rearrange("p (l q) -> p l q", q=512)
                if causal and k0 >= qb0:
                    # diagonal tile: 0/-1e30 triangle on both heads
                    nc.vector.tensor_add(
                        spv[:, :, c0 : c0 + P],
                        spv[:, :, c0 : c0 + P],
                        tri2_sb[:].rearrange("p (l q) -> p l q", q=P),
                    )
                ex = expp.tile([P, 1024], BF16, tag="exp", bufs=EXBUFS)
                exv = ex[:].rearrange("p (l q) -> p l q", q=512)
                nc.scalar.activation(
                    exv[:, :, c0:512], spv[:, :, c0:512], Exp, scale=SCALE
                )
                exs.append((mk, exv, c0))
            return exs

        def emit_attended(hp, att_ps, items, mk_hi):
            # attended (128-mode), accumulated over mk; emitted behind
            # scores/exp so exps are done when the schedule reaches these
            # MMs, and 64/128 tile-mode runs stay long.
            for mk, exv, c0 in items:
                for hl in (0, 1):
                    h = 2 * hp + hl
                    nc.tensor.matmul(
                        att_ps[hl][0 : DH + 1, c0:512],
                        v_sb[mk][:, h * (DH + 1) : (h + 1) * (DH + 1)],
                        exv[:, hl, c0:512],
                        start=(mk == 0),
                        stop=(mk == mk_hi - 1),
                        skip_group_check=True,
                    )

        def emit_norm(hp, qbl, att_ps):
            qb0 = qbl * 512
            # normalize: evacuate PSUM fast, then recip+broadcast in SBUF
            for hl in (0, 1):
                    au = attup.tile([P, 512], BF16, tag="attu")
                    nc.vector.tensor_copy(au[0 : DH + 1, :], att_ps[hl][0 : DH + 1, :])
                    # spread den over 32 lanes via a DRAM bounce (SBUF APs
                    # cannot repartition or stride-0 broadcast), reciprocal,
                    # linearize back, broadcast-read to DH partitions.  The
                    # chain DMAs issue from the otherwise-idle gpsimd queue
                    # so their inter-hop waits don't head-of-line-block the
                    # sync queue's bulk x/w/out transfers.
                    dend = dramp.tile([1, 512], BF16, tag="dend")
                    nc.sync.dma_start(dend[:], au[DH : DH + 1, :])
                    denp = smallp.tile([32, 16], BF16, tag="denp")
                    nc.sync.dma_start(
                        denp[:], dend[:].rearrange("o (p c) -> (o p) c", c=16)
                    )
                    with nc.allow_low_precision(reason="softmax denom recip in bf16"):
                        nc.vector.reciprocal(denp[:], denp[:])
                    dend2 = dramp.tile([1, 512], BF16, tag="dend2")
                    nc.sync.dma_start(
                        dend2[:].rearrange("o (p c) -> (o p) c", c=16), denp[:]
                    )
                    rep = smallp.tile([DH, 512], BF16, tag="rep")
                    nc.sync.dma_start(rep[:], dend2[:].broadcast_to([DH, 512]))
                    nc.vector.tensor_mul(
                        att_sb[hp][hl * DH : (hl + 1) * DH, qb0 : qb0 + 512],
                        au[0:DH, :],
                        rep[:],
                    )

        def emit_attention(hp, qbls, mid=None):
            if mid is None:
                for qbl in qbls:
                    mk_hi = hi_of(qbl)
                    att_ps = alloc_att_ps(hp, qbl)
                    pending = []
                    for ck0 in range(0, mk_hi, EXCHUNK):
                        exs = emit_scores_exp(
                            hp, qbl, range(ck0, min(ck0 + EXCHUNK, mk_hi))
                        )
                        emit_attended(hp, att_ps, pending, mk_hi)
                        pending = exs
                    emit_attended(hp, att_ps, pending, mk_hi)
                    emit_norm(hp, qbl, att_ps)
            else:
                # split mode: all scores/exp first, then mid() (e.g. the v
                # projections), then the deferred attended + normalization.
                saved = []
                for qbl in qbls:
                    mk_hi = hi_of(qbl)
                    att_ps = alloc_att_ps(hp, qbl)
                    exs = emit_scores_exp(hp, qbl, range(mk_hi))
                    saved.append((qbl, att_ps, exs, mk_hi))
                mid()
                for qbl, att_ps, exs, mk_hi in saved:
                    emit_attended(hp, att_ps, exs, mk_hi)
                    emit_norm(hp, qbl, att_ps)

        def emit_outproj(sts):
            for st in sts:
                ot = outp.tile([P, D], F32, tag="out")
                for eb in range(2):
                    po = psum.tile([P, 512], F32, tag="pqkv", bufs=2, name=f"po{st}_{eb}")
                    for kt in range(ET):
                        nc.tensor.matmul(
                            po[:],
                            att_sb[kt][:, st * P : (st + 1) * P],
                            wot_sb[kt][:, eb * 512 : eb * 512 + 512],
                            start=(kt == 0),
                            stop=(kt == ET - 1),
                        )
                    if eb == 0:
                        nc.vector.tensor_copy(ot[:, 0:512], po[:])
                    else:
                        nc.scalar.copy(ot[:, 512:1024], po[:])
                    nc.sync.dma_start(
                        out_d[st * P : (st + 1) * P, eb * 512 : eb * 512 + 512],
                        ot[:, eb * 512 : eb * 512 + 512],
                    )

        # ---- interleaved emission: attention is emitted at higher priority
        # than the remaining qkv m-tiles (the scheduler fills PE stalls with
        # qkv work by readiness anyway), so ACT starts exp as early as
        # possible and never starves -- the exp stream end gates the tail.
        # attn0's scores/exp need only qt/kt m=0; its attended MMs wait on v
        # via data deps while exp runs ahead (EXBUFS covers ~2 chunks). ----
        emit_qk(0)
        emit_v()
        emit_attention(0, [0, 1])
        emit_qk(1)
        emit_attention(0, [2, 3])
        emit_attention(1, [0, 1])
        emit_qk(2)
        emit_attention(1, [2, 3])
        emit_attention(2, [0, 1])
        emit_qk(3)
        emit_attention(2, [2, 3])
        # hp3 processes its shortest q-block (qbl0, 4 mk) LAST so the final
        # softmax-denominator DMA chain overlaps the outproj of group 3,
        # and the kernel ends on outproj group 0 whose inputs are ready.
        emit_attention(3, [1])
        emit_outproj(range(4, 8))
        emit_attention(3, [2])
        emit_outproj(range(8, 12))
        emit_attention(3, [3])
        emit_outproj(range(12, 16))
        emit_attention(3, [0])
        emit_outproj(range(0, 4))

    if split_waits:
        _split_sync_waits(nc)
    return nc


def get_program(mode, split_waits=True):
    key = (mode, split_waits)
    if key not in _PROGRAM_CACHE:
        _PROGRAM_CACHE[key] = build_program(mode, split_waits)
    return _PROGRAM_CACHE[key]


def _detect_mode(mask):
    m = np.asarray(mask)
    if np.array_equal(m != 0, np.tril(np.ones(m.shape, dtype=bool))):
        return "causal"
    if np.all(m != 0):
        return "full"
    raise NotImplementedError("only causal (tril) or all-ones masks supported")


def make_tri2(mode):
    """Additive diagonal-tile mask, doubled along free dim for the two
    heads of a pair: 0 on/above the in-tile diagonal (q >= k, valid),
    -1e30 below (masked)."""
    if mode != "causal":
        return np.zeros((P, 2 * P), dtype=np.float32)
    kk = np.arange(P)[:, None]
    cc = np.arange(P)[None, :]
    tri = np.where(cc >= kk, 0.0, NEG).astype(np.float32)
    return np.concatenate([tri, tri], axis=1)


def make_in_maps(x, Wq, bq, Wk, Wv, bv, Wo, mode):
    bf = mybir.dt.np(BF16)
    x = np.asarray(x, dtype=np.float32)
    B = x.shape[0]
    tri2 = make_tri2(mode)
    xts = [np.ascontiguousarray(x[b].T).astype(bf) for b in range(B)]
    in_maps = []
    for c in range(NCORES):
        b, hg = divmod(c, 2)
        sl = slice(hg * EL, (hg + 1) * EL)
        in_maps.append(
            {
                "xt": xts[b],
                "wqt": np.ascontiguousarray(
                    np.asarray(Wq, np.float32)[sl, :].T
                ).astype(bf),
                "wkt": np.ascontiguousarray(
                    np.asarray(Wk, np.float32)[sl, :].T
                ).astype(bf),
                "wvt": np.ascontiguousarray(
                    np.asarray(Wv, np.float32)[sl, :].T
                ).astype(bf),
                "wot": np.ascontiguousarray(
                    np.asarray(Wo, np.float32)[:, sl].T
                ).astype(bf),
                "bq": np.ascontiguousarray(
                    np.asarray(bq, np.float32)[sl].reshape(EL, 1)
                ),
                "bvrep": np.ascontiguousarray(
                    np.broadcast_to(np.asarray(bv, np.float32)[sl], (P, EL))
                ).astype(bf),
                "tri2": tri2,
            }
        )
    return in_maps


def run(x, mask, Wq, bq, Wk, bk, Wv, bv, Wo, bo, trace=False, **spmd_kwargs):
    """Returns (full_output, BassKernelResults)."""
    from concourse.bass_utils import run_bass_kernel_spmd

    mode = _detect_mode(mask)
    nc = get_program(mode)
    in_maps = make_in_maps(x, Wq, bq, Wk, Wv, bv, Wo, mode)

    res = run_bass_kernel_spmd(
        nc, in_maps, core_ids=list(range(NCORES)), trace=trace, **spmd_kwargs
    )
    B = np.asarray(x).shape[0]
    out = np.empty((B, S, D), dtype=np.float32)
    bo = np.asarray(bo, np.float32)
    for b in range(B):
        out[b] = res.results[2 * b]["out"] + res.results[2 * b + 1]["out"] + bo
    return out, res


def kernel(x, mask, Wq, bq, Wk, bk, Wv, bv, Wo, bo):
    out, _ = run(x, mask, Wq, bq, Wk, bk, Wv, bv, Wo, bo)
    return out
